# revision 3
# baseline (speedup 1.0000x reference)
"""Locality self-attention (dense softmax attention) for Trainium2, 8 cores.

Math per (b, h) pair:  out = softmax(Q K^T * exp(scale), axis=key) @ V
Shapes: B=2, L=2048, H=8, D=64, fp32.  16 (b,h) pairs -> 2 per core.

Device layout (per pair, all "transposed" so the PV matmul needs no
on-chip transpose of the score matrix):
  S^T[s, l]   = K Q^T             (PE, fp16 inputs, fp32 PSUM)
  E^T         = exp(S^T)          (ACT, PSUM -> SBUF fp16)  <- bottleneck
  Oa^T[d+1,l] = [V | 1]^T E^T     (PE, accumulated over s in PSUM;
                                   row 64 = softmax denominators)
  O[l, d]     = Oa^T[:64].T / Oa^T[64]   (PE transpose + DVE recip/mul)
"""

import sys

sys.path.insert(0, "/opt/trn_rl_repo")

import numpy as np

B, L, H, D = 2, 2048, 8, 64
NCORES = 8
PAIRS_PER_CORE = (B * H) // NCORES  # 2
NST = L // 128  # 16 s-tiles per pair
NLC = L // 512  # 4 l-chunks of 512

_CACHE = {}
last_results = None  # BassKernelResults of the most recent run (for test.py)


def _build_nc():
    import concourse.tile as tile
    import concourse.mybir as mybir
    from concourse import bacc
    from concourse.masks import make_identity

    f32 = mybir.dt.float32
    f16 = mybir.dt.float16
    Exp = mybir.ActivationFunctionType.Exp

    nc = bacc.Bacc("TRN2", target_bir_lowering=False, name="attn8")
    qT_d = nc.dram_tensor("qT", [PAIRS_PER_CORE, D, L], f32, kind="ExternalInput").ap()
    kT_d = nc.dram_tensor("kT", [PAIRS_PER_CORE, D, L], f32, kind="ExternalInput").ap()
    v_d = nc.dram_tensor("v", [PAIRS_PER_CORE, L, D], f32, kind="ExternalInput").ap()
    sc_d = nc.dram_tensor("scale", [1, 1], f32, kind="ExternalInput").ap()
    o_d = nc.dram_tensor("o", [PAIRS_PER_CORE, L, D], f32, kind="ExternalOutput").ap()

    with tile.TileContext(nc) as tc:
        with (
            tc.tile_pool(name="singles", bufs=1) as singles,
            tc.tile_pool(name="stage", bufs=2) as stage,
            tc.tile_pool(name="qk", bufs=4) as qkp,
            tc.tile_pool(name="vt", bufs=2) as vtp,
            tc.tile_pool(name="et", bufs=2) as etp,
            tc.tile_pool(name="oc", bufs=1) as ocp,
            tc.tile_pool(name="osb", bufs=2) as osbp,
            tc.tile_pool(name="st", bufs=2, space="PSUM") as stp,
            tc.tile_pool(name="big", bufs=1, space="PSUM") as bigp,
        ):
            # exp(scale), broadcast to all 128 partitions
            s_sb = singles.tile([128, 1], f32)
            nc.sync.dma_start(out=s_sb, in_=sc_d.to_broadcast((128, 1)))
            s_exp = singles.tile([128, 1], f32)
            nc.scalar.activation(out=s_exp, in_=s_sb, func=Exp)

            ident = singles.tile([128, 128], f32)
            make_identity(nc, ident)

            for p in range(PAIRS_PER_CORE):
                # ---- load + cast inputs for this pair ----
                q_f = stage.tile([D, L], f32, tag="stage")
                nc.sync.dma_start(out=q_f, in_=qT_d[p])
                qh = qkp.tile([D, L], f16, tag="qk")
                # fold exp(scale) into q during the fp32->fp16 cast
                nc.vector.tensor_scalar_mul(out=qh, in0=q_f, scalar1=s_exp[:D])

                k_f = stage.tile([D, L], f32, tag="stage")
                nc.sync.dma_start(out=k_f, in_=kT_d[p])
                kh = qkp.tile([D, L], f16, tag="qk")
                nc.vector.tensor_copy(kh, k_f)

                v_f = stage.tile([128, NST, D], f32, tag="stage")
                nc.sync.dma_start(
                    out=v_f, in_=v_d[p].rearrange("(c p) d -> p c d", p=128)
                )
                vt = vtp.tile([128, NST, D + 1], f16, tag="vt")
                nc.vector.memset(vt, 1.0)  # ones column (col D) = rowsum probe
                nc.vector.tensor_copy(vt[:, :, 0:D], v_f)

                et = etp.tile([128, NST, L], f16, tag="et")
                oacc = bigp.tile([D + 1, L], f32, tag="big")

                # ---- main pipeline over s-tiles ----
                for i in range(NST):
                    khs = kh[:, i * 128 : (i + 1) * 128]
                    for half in range(2):
                        st = stp.tile([128, 1024], f32, tag="st")
                        for jj in range(2):
                            j = half * 2 + jj
                            nc.tensor.matmul(
                                out=st[:, jj * 512 : (jj + 1) * 512],
                                lhsT=khs,
                                rhs=qh[:, j * 512 : (j + 1) * 512],
                                start=True,
                                stop=True,
                            )
                        nc.scalar.activation(
                            out=et[:, i, half * 1024 : (half + 1) * 1024],
                            in_=st,
                            func=Exp,
                        )
                    for j in range(NLC):
                        nc.tensor.matmul(
                            out=oacc[:, j * 512 : (j + 1) * 512],
                            lhsT=vt[:, i, :],
                            rhs=et[:, i, j * 512 : (j + 1) * 512],
                            start=(i == 0),
                            stop=(i == NST - 1),
                        )

                # ---- finalize: normalize + transpose back ----
                oc = ocp.tile([D + 1, L], f32, tag="oc")
                nc.vector.tensor_copy(oc[:, 0:1024], oacc[:, 0:1024])
                nc.vector.tensor_copy(oc[:, 1024:2048], oacc[:, 1024:2048])

                tr = bigp.tile([128, NST, 128], f32, tag="big")
                rinv = osbp.tile([128, NST], f32, tag="rinv")
                osb = osbp.tile([128, NST, D], f32, tag="osb")
                for c in range(NST):
                    nc.tensor.transpose(
                        out=tr[:, c, 0 : D + 1],
                        in_=oc[:, c * 128 : (c + 1) * 128],
                        identity=ident[: D + 1, : D + 1],
                    )
                    nc.vector.reciprocal(
                        out=rinv[:, c : c + 1], in_=tr[:, c, D : D + 1]
                    )
                    nc.vector.tensor_scalar_mul(
                        out=osb[:, c, :],
                        in0=tr[:, c, 0:D],
                        scalar1=rinv[:, c : c + 1],
                    )
                nc.sync.dma_start(
                    out=o_d[p].rearrange("(c p) d -> p c d", p=128), in_=osb
                )

    nc.compile()
    return nc


def _get_nc():
    if "nc" not in _CACHE:
        _CACHE["nc"] = _build_nc()
    return _CACHE["nc"]


def kernel(query, key, value, scale, _trace=False, **trace_kwargs):
    global last_results
    from concourse.bass_utils import run_bass_kernel_spmd

    nc = _get_nc()

    query = np.asarray(query, np.float32)
    key = np.asarray(key, np.float32)
    value = np.asarray(value, np.float32)
    sc = np.asarray(scale, np.float32).reshape(1, 1)

    in_maps = []
    pair_idx = []
    for c in range(NCORES):
        idxs = [c * PAIRS_PER_CORE + j for j in range(PAIRS_PER_CORE)]
        pair_idx.append(idxs)
        qT = np.ascontiguousarray(
            np.stack([query[i // H, :, i % H, :].T for i in idxs])
        )
        kT = np.ascontiguousarray(np.stack([key[i // H, :, i % H, :].T for i in idxs]))
        vv = np.ascontiguousarray(np.stack([value[i // H, :, i % H, :] for i in idxs]))
        in_maps.append({"qT": qT, "kT": kT, "v": vv, "scale": sc})

    try:
        res = run_bass_kernel_spmd(
            nc,
            in_maps,
            core_ids=list(range(NCORES)),
            trace=_trace,
            **trace_kwargs,
        )
    except ModuleNotFoundError:
        # axon NTFF profiling hook unavailable in this container
        res = run_bass_kernel_spmd(
            nc, in_maps, core_ids=list(range(NCORES)), trace=False
        )
    last_results = res

    out = np.empty((B, L, H, D), np.float32)
    for c in range(NCORES):
        for j, i in enumerate(pair_idx[c]):
            out[i // H, :, i % H, :] = res.results[c]["o"][j]
    return out
